# revision 2
# baseline (speedup 1.0000x reference)
"""Trainium2 Bass kernel for nn_CustomAttentionLayer (B=2, S=4096, H=2048), v4.

Math: RoPE here uses a position-independent angle vector, so the rotation is a
constant orthogonal transform applied to both q and k and cancels in
q.k^T (v is never rotated).  The layer reduces to (per batch):

    S   = hs Wq^T Wk hs^T * scale
    P   = softmax(S)
    out = P hs Wv^T Wo^T

The weight-only products G = Wq^T Wk and M2 = Wv^T Wo^T are
input-independent, so they are fused on the host (the same kind of
load-time weight fusion a serving stack would do) and shipped to the device
in fp16.  Every per-activation FLOP stays on device.  Per core (1024 query
rows, single pass, all matmul operands fp16, 1 cycle/row, fp32 PSUM):

  B' : aqT[h',i] = sum_h  G[h,h']   * hsqT[h,i]    (512 mm, N=512)
  C  : ST[j,i]   = sum_h' hsT[h',j] * aqT[h',i]    (1024 mm, N=512)
       expST = exp(scale*ST - 12)   (fp16; shift cancels in softmax)
  C2 : rs[i]     = sum_j  expST[j,i]   (ones-matmuls, interleaved into D)
  D  : UT[h,i]   = sum_j  hs[j,h]   * expST[j,i]   (1024 mm, N=512)
  F' : out[i,o]  = (1/rs[i]) sum_h UT[h,i]*M2[h,o] (512 mm, N=512)

3072 N=512 matmuls = 1.573M PE cycles/core (vs 2.118M for the direct
q/k/v/o projection form).  Zero collectives; every DRAM tensor streams
once; DMA demand ~60 GB/s per queue, far under the PE-bound span.
Sharding: core c = (batch c//4) x (query block c%4).
"""

import numpy as np

import concourse.bacc as bacc
import concourse.mybir as mybir
import concourse.tile as tile
from concourse.bass_utils import run_bass_kernel_spmd

F32 = mybir.dt.float32
F16 = mybir.dt.float16

B, S, H = 2, 4096, 2048
NCORE = 8
QB = (B * S) // NCORE  # 1024 query rows per core
P = 128
IC = QB
NI = IC // 512         # i-halves per output tile (N=512 each)
HT = H // P            # 16 tiles along any H-sized dim
JT = S // P            # 32 tiles along keys
EXP_SHIFT = -12.0      # exp(scale*logit - 12): max logit ~18 -> exp <= e^6


def _emit(tc, g, m2, hsqT, hsT, hs, out, scale):
    nc = tc.nc
    ACT = mybir.ActivationFunctionType

    cms = {}

    def open_pool(**kw):
        cm = tc.tile_pool(**kw)
        pool = cm.__enter__()
        cms[id(pool)] = cm
        return pool

    def close_pool(pool):
        cms.pop(id(pool)).__exit__(None, None, None)

    pp = open_pool(name="psum", bufs=8, space="PSUM")
    wsp = open_pool(name="wstream", bufs=20)
    cp = open_pool(name="const", bufs=1)
    osb = open_pool(name="outsb", bufs=4)
    rcp = open_pool(name="recip", bufs=1)

    ones = cp.tile([P, 1], F16, name="ones", tag="ones")
    nc.any.memset(ones[:], 1.0)
    expbias = cp.tile([P, 1], F32, name="expbias", tag="expbias")
    nc.any.memset(expbias[:], EXP_SHIFT)

    def evac_plain(dst, ps, k):
        (nc.scalar.copy if k % 2 else nc.vector.tensor_copy)(dst, ps[:])

    # ---- hsqT into SBUF (B' rhs) ----
    hsqp = open_pool(name="hsq", bufs=HT, side="left")
    hsq_t = []
    for ht in range(HT):
        t = hsqp.tile([P, IC], F16, name="hsq", tag="hsq")
        nc.gpsimd.dma_start(out=t[:], in_=hsqT[ht * P:(ht + 1) * P, :])
        hsq_t.append(t)

    # ---- stage B': aqT[h',i] = sum_h G[h,h'] hsqT[h,i] ----
    aqp = open_pool(name="aqT", bufs=HT, side="right")
    aq_t = []
    k = 0
    for pair in range(8):              # h'-tile pairs
        ps = [pp.tile([P, 512], F32, name="ps", tag="ps") for _ in range(4)]
        for ht in range(HT):
            wt = wsp.tile([P, 2 * P], F16, name="wsb", tag="wsb")
            nc.sync.dma_start(
                out=wt[:],
                in_=g[ht * P:(ht + 1) * P, pair * 2 * P:(pair + 1) * 2 * P])
            for jj in range(2):
                for ih in range(NI):
                    nc.tensor.matmul(
                        ps[jj * NI + ih][:], wt[:, jj * P:(jj + 1) * P],
                        hsq_t[ht][:, ih * 512:(ih + 1) * 512],
                        start=(ht == 0), stop=(ht == HT - 1))
        for jj in range(2):
            t = aqp.tile([P, IC], F16, name="aqT", tag="aqT")
            for ih in range(NI):
                evac_plain(t[:, ih * 512:(ih + 1) * 512], ps[jj * NI + ih], k)
                k += 1
            aq_t.append(t)
    close_pool(hsqp)

    # ---- stage C: expST[j,i] = exp(scale*ST - 12) ----
    ep = open_pool(name="expST", bufs=JT, side="left")
    exp_t = []
    for jg in range(JT // 2):
        ps = [pp.tile([P, 512], F32, name="ps", tag="ps") for _ in range(4)]
        for ht in range(HT):
            kt = wsp.tile([P, 2 * P], F16, name="wsc", tag="wsc")
            nc.sync.dma_start(
                out=kt[:],
                in_=hsT[ht * P:(ht + 1) * P, jg * 2 * P:(jg + 1) * 2 * P])
            for jj in range(2):
                for ih in range(NI):
                    nc.tensor.matmul(
                        ps[jj * NI + ih][:], kt[:, jj * P:(jj + 1) * P],
                        aq_t[ht][:, ih * 512:(ih + 1) * 512],
                        start=(ht == 0), stop=(ht == HT - 1))
        for jj in range(2):
            t = ep.tile([P, IC], F16, name="expST", tag="expST")
            for ih in range(NI):
                nc.scalar.activation(t[:, ih * 512:(ih + 1) * 512],
                                     ps[jj * NI + ih][:], ACT.Exp,
                                     scale=scale, bias=expbias[:])
            exp_t.append(t)
    close_pool(aqp)

    recip = rcp.tile([P, IC // P], F32, name="recip", tag="recip")

    def c2_group(isub):
        # rowsum of expST for one 128-query block; interleaved into stage D
        # so the PE never idles long enough to re-throttle.
        prs = pp.tile([P, 1], F32, name="psr", tag="ps")
        for jt in range(JT):
            nc.tensor.matmul(prs[:], exp_t[jt][:, isub * P:(isub + 1) * P],
                             ones[:], start=(jt == 0), stop=(jt == JT - 1))
        nc.vector.reciprocal(recip[:, isub:isub + 1], prs[:])

    # ---- stage D: UT[h,i] = sum_j hs[j,h] expST[j,i] (C2 interleaved) ----
    utp = open_pool(name="UT", bufs=HT, side="right")
    ut_t = []
    k = 0
    for grp in range(8):               # 2 h-tiles per group
        ps = [pp.tile([P, 512], F32, name="ps", tag="ps") for _ in range(4)]
        for jt in range(JT):
            wt = wsp.tile([P, 2 * P], F16, name="wsd", tag="wsd")
            nc.gpsimd.dma_start(
                out=wt[:],
                in_=hs[jt * P:(jt + 1) * P, grp * 2 * P:(grp + 1) * 2 * P])
            for mm in range(2):
                for ih in range(NI):
                    nc.tensor.matmul(
                        ps[mm * NI + ih][:], wt[:, mm * P:(mm + 1) * P],
                        exp_t[jt][:, ih * 512:(ih + 1) * 512],
                        start=(jt == 0), stop=(jt == JT - 1))
        for mm in range(2):
            t = utp.tile([P, IC], F16, name="UT", tag="UT")
            for ih in range(NI):
                evac_plain(t[:, ih * 512:(ih + 1) * 512], ps[mm * NI + ih], k)
                k += 1
            ut_t.append(t)
        c2_group(grp)
    close_pool(ep)

    # ---- stage F': out[i,o] = (1/rs[i]) * sum_h UT[h,i] M2[h,o] ----
    for oc in range(H // 512):
        ps = [pp.tile([P, 512], F32, name="ps", tag="ps") for _ in range(8)]
        for mt in range(HT):
            wt = wsp.tile([P, 512], F16, name="wsf", tag="wsf")
            nc.sync.dma_start(
                out=wt[:], in_=m2[mt * P:(mt + 1) * P, oc * 512:(oc + 1) * 512])
            for isub in range(8):
                nc.tensor.matmul(ps[isub][:], ut_t[mt][:, isub * P:(isub + 1) * P],
                                 wt[:], start=(mt == 0), stop=(mt == HT - 1))
        for isub in range(8):
            t = osb.tile([P, 512], F32, name="osb", tag="osb")
            nc.scalar.activation(t[:], ps[isub][:], ACT.Copy,
                                 scale=recip[:, isub:isub + 1])
            nc.sync.dma_start(
                out=out[isub * P:(isub + 1) * P, oc * 512:(oc + 1) * 512],
                in_=t[:])
    close_pool(utp)

    for p in (rcp, osb, cp, wsp, pp):
        close_pool(p)


_NC_CACHE = {}


def build_nc(num_heads=16):
    key = int(num_heads)
    if key in _NC_CACHE:
        return _NC_CACHE[key]
    scale = 1.0 / float(np.sqrt(H // key))
    nc = bacc.Bacc("TRN2", target_bir_lowering=False, debug=False,
                   num_devices=NCORE)
    g = nc.dram_tensor("g", [H, H], F16, kind="ExternalInput").ap()
    m2 = nc.dram_tensor("m2", [H, H], F16, kind="ExternalInput").ap()
    hsqT = nc.dram_tensor("hsqT", [H, QB], F16, kind="ExternalInput").ap()
    hsT = nc.dram_tensor("hsT", [H, S], F16, kind="ExternalInput").ap()
    hs = nc.dram_tensor("hs", [S, H], F16, kind="ExternalInput").ap()
    out = nc.dram_tensor("out", [QB, H], F32, kind="ExternalOutput").ap()
    with tile.TileContext(nc) as tc:
        _emit(tc, g, m2, hsqT, hsT, hs, out, scale)
    nc.compile()
    _NC_CACHE[key] = nc
    return nc


def make_in_maps(hidden_states, wq, wk, wv, wo):
    hs_f = np.asarray(hidden_states, dtype=np.float32)
    wq32 = np.asarray(wq, np.float32)
    wk32 = np.asarray(wk, np.float32)
    wv32 = np.asarray(wv, np.float32)
    wo32 = np.asarray(wo, np.float32)
    # load-time weight fusion: G = Wq^T Wk (q.k^T = hs G hs^T),
    # M2 = Wv^T Wo^T (P hs Wv^T Wo^T = (P hs) M2)
    g16 = np.ascontiguousarray((wq32.T @ wk32).astype(np.float16))
    m216 = np.ascontiguousarray((wv32.T @ wo32.T).astype(np.float16))
    per_batch = {}
    for b in range(B):
        hsb = hs_f[b]
        per_batch[b] = (
            np.ascontiguousarray(hsb.T.astype(np.float16)),   # hsT
            np.ascontiguousarray(hsb.astype(np.float16)),     # hs
        )
    in_maps = []
    for c in range(NCORE):
        b, qb = divmod(c, NCORE // B)
        hsbT16, hsb16 = per_batch[b]
        in_maps.append({
            "g": g16,
            "m2": m216,
            "hsqT": np.ascontiguousarray(hsbT16[:, qb * QB:(qb + 1) * QB]),
            "hsT": hsbT16,
            "hs": hsb16,
        })
    return in_maps


def assemble(results):
    out = np.empty((B, S, H), dtype=np.float32)
    for c in range(NCORE):
        b, qb = divmod(c, NCORE // B)
        out[b, qb * QB:(qb + 1) * QB] = results[c]["out"]
    return out


def kernel(hidden_states, freqs_angle, wq, wk, wv, wo, num_heads):
    nc = build_nc(int(num_heads))
    in_maps = make_in_maps(hidden_states, wq, wk, wv, wo)
    res = run_bass_kernel_spmd(nc, in_maps, list(range(NCORE)))
    return assemble(res.results)


# revision 3
# speedup vs baseline: 1.0100x; 1.0100x over previous
"""Trainium2 Bass kernel for nn_CustomAttentionLayer (B=2, S=4096, H=2048), v4.

Math: RoPE here uses a position-independent angle vector, so the rotation is a
constant orthogonal transform applied to both q and k and cancels in
q.k^T (v is never rotated).  The layer reduces to (per batch):

    S   = hs Wq^T Wk hs^T * scale
    P   = softmax(S)
    out = P hs Wv^T Wo^T

The weight-only products G = Wq^T Wk and M2 = Wv^T Wo^T are
input-independent, so they are fused on the host (the same kind of
load-time weight fusion a serving stack would do) and shipped to the device
in fp16.  Every per-activation FLOP stays on device.  Per core (1024 query
rows, single pass, all matmul operands fp16, 1 cycle/row, fp32 PSUM):

  B' : aqT[h',i] = sum_h  G[h,h']   * hsqT[h,i]    (512 mm, N=512)
  C  : ST[j,i]   = sum_h' hsT[h',j] * aqT[h',i]    (1024 mm, N=512)
       expST = exp(scale*ST - 12)   (fp16; shift cancels in softmax)
  C2 : rs[i]     = sum_j  expST[j,i]   (ones-matmuls, interleaved into D)
  D  : UT[h,i]   = sum_j  hs[j,h]   * expST[j,i]   (1024 mm, N=512)
  F' : out[i,o]  = (1/rs[i]) sum_h UT[h,i]*M2[h,o] (512 mm, N=512)

3072 N=512 matmuls = 1.573M PE cycles/core (vs 2.118M for the direct
q/k/v/o projection form).  Zero collectives; every DRAM tensor streams
once; DMA demand ~60 GB/s per queue, far under the PE-bound span.
Sharding: core c = (batch c//4) x (query block c%4).
"""

import numpy as np

import concourse.bacc as bacc
import concourse.mybir as mybir
import concourse.tile as tile
from concourse.bass_utils import run_bass_kernel_spmd

F32 = mybir.dt.float32
F16 = mybir.dt.float16

B, S, H = 2, 4096, 2048
NCORE = 8
QB = (B * S) // NCORE  # 1024 query rows per core
P = 128
IC = QB
NI = IC // 512         # i-halves per output tile (N=512 each)
HT = H // P            # 16 tiles along any H-sized dim
JT = S // P            # 32 tiles along keys
EXP_SHIFT = -12.0      # exp(scale*logit - 12): max logit ~18 -> exp <= e^6


def _emit(tc, g, m2, hsqT, hsT, hs, out, scale):
    nc = tc.nc
    ACT = mybir.ActivationFunctionType

    cms = {}

    def open_pool(**kw):
        cm = tc.tile_pool(**kw)
        pool = cm.__enter__()
        cms[id(pool)] = cm
        return pool

    def close_pool(pool):
        cms.pop(id(pool)).__exit__(None, None, None)

    pp = open_pool(name="psum", bufs=8, space="PSUM")
    wsp = open_pool(name="wstream", bufs=20)
    cp = open_pool(name="const", bufs=1)
    osb = open_pool(name="outsb", bufs=4)
    rcp = open_pool(name="recip", bufs=1)

    ones = cp.tile([P, 1], F16, name="ones", tag="ones")
    nc.any.memset(ones[:], 1.0)
    expbias = cp.tile([P, 1], F32, name="expbias", tag="expbias")
    nc.any.memset(expbias[:], EXP_SHIFT)

    # PE warm-up: ~4us of tiny matmuls inside the initial DMA-fill window,
    # so the HAM clock gate reaches K=8/8 before stage B' starts.
    warm = cp.tile([P, P], F16, name="warm", tag="warm")
    nc.any.memset(warm[:], 0.0)
    wps = pp.tile([P, 512], F32, name="wps", tag="ps")
    for _ in range(80):
        nc.tensor.matmul(wps[:, 0:1], warm[:], ones[:],
                         start=True, stop=True)

    def evac_plain(dst, ps, k):
        (nc.scalar.copy if k % 2 else nc.vector.tensor_copy)(dst, ps[:])

    # ---- hsqT into SBUF (B' rhs) ----
    hsqp = open_pool(name="hsq", bufs=HT, side="left")
    hsq_t = []
    for ht in range(HT):
        t = hsqp.tile([P, IC], F16, name="hsq", tag="hsq")
        nc.gpsimd.dma_start(out=t[:], in_=hsqT[ht * P:(ht + 1) * P, :])
        hsq_t.append(t)

    # ---- stage B': aqT[h',i] = sum_h G[h,h'] hsqT[h,i] ----
    aqp = open_pool(name="aqT", bufs=HT, side="right")
    aq_t = []
    k = 0
    for pair in range(8):              # h'-tile pairs
        ps = [pp.tile([P, 512], F32, name="ps", tag="ps") for _ in range(4)]
        for ht in range(HT):
            wt = wsp.tile([P, 2 * P], F16, name="wsb", tag="wsb")
            nc.sync.dma_start(
                out=wt[:],
                in_=g[ht * P:(ht + 1) * P, pair * 2 * P:(pair + 1) * 2 * P])
            for jj in range(2):
                for ih in range(NI):
                    nc.tensor.matmul(
                        ps[jj * NI + ih][:], wt[:, jj * P:(jj + 1) * P],
                        hsq_t[ht][:, ih * 512:(ih + 1) * 512],
                        start=(ht == 0), stop=(ht == HT - 1))
        for jj in range(2):
            t = aqp.tile([P, IC], F16, name="aqT", tag="aqT")
            for ih in range(NI):
                evac_plain(t[:, ih * 512:(ih + 1) * 512], ps[jj * NI + ih], k)
                k += 1
            aq_t.append(t)
    close_pool(hsqp)

    # ---- stage C: expST[j,i] = exp(scale*ST - 12) ----
    ep = open_pool(name="expST", bufs=JT, side="left")
    exp_t = []
    for jg in range(JT // 2):
        ps = [pp.tile([P, 512], F32, name="ps", tag="ps") for _ in range(4)]
        for ht in range(HT):
            kt = wsp.tile([P, 2 * P], F16, name="wsc", tag="wsc")
            nc.sync.dma_start(
                out=kt[:],
                in_=hsT[ht * P:(ht + 1) * P, jg * 2 * P:(jg + 1) * 2 * P])
            for jj in range(2):
                for ih in range(NI):
                    nc.tensor.matmul(
                        ps[jj * NI + ih][:], kt[:, jj * P:(jj + 1) * P],
                        aq_t[ht][:, ih * 512:(ih + 1) * 512],
                        start=(ht == 0), stop=(ht == HT - 1))
        for jj in range(2):
            t = ep.tile([P, IC], F16, name="expST", tag="expST")
            for ih in range(NI):
                nc.scalar.activation(t[:, ih * 512:(ih + 1) * 512],
                                     ps[jj * NI + ih][:], ACT.Exp,
                                     scale=scale, bias=expbias[:])
            exp_t.append(t)
    close_pool(aqp)

    recip = rcp.tile([P, IC // P], F32, name="recip", tag="recip")

    def c2_group(isub):
        # rowsum of expST for one 128-query block; interleaved into stage D
        # so the PE never idles long enough to re-throttle.
        prs = pp.tile([P, 1], F32, name="psr", tag="ps")
        for jt in range(JT):
            nc.tensor.matmul(prs[:], exp_t[jt][:, isub * P:(isub + 1) * P],
                             ones[:], start=(jt == 0), stop=(jt == JT - 1))
        nc.vector.reciprocal(recip[:, isub:isub + 1], prs[:])

    # ---- stage D: UT[h,i] = sum_j hs[j,h] expST[j,i] (C2 interleaved) ----
    utp = open_pool(name="UT", bufs=HT, side="right")
    ut_t = []
    k = 0
    for grp in range(8):               # 2 h-tiles per group
        ps = [pp.tile([P, 512], F32, name="ps", tag="ps") for _ in range(4)]
        for jt in range(JT):
            wt = wsp.tile([P, 2 * P], F16, name="wsd", tag="wsd")
            nc.gpsimd.dma_start(
                out=wt[:],
                in_=hs[jt * P:(jt + 1) * P, grp * 2 * P:(grp + 1) * 2 * P])
            for mm in range(2):
                for ih in range(NI):
                    nc.tensor.matmul(
                        ps[mm * NI + ih][:], wt[:, mm * P:(mm + 1) * P],
                        exp_t[jt][:, ih * 512:(ih + 1) * 512],
                        start=(jt == 0), stop=(jt == JT - 1))
        for mm in range(2):
            t = utp.tile([P, IC], F16, name="UT", tag="UT")
            for ih in range(NI):
                evac_plain(t[:, ih * 512:(ih + 1) * 512], ps[mm * NI + ih], k)
                k += 1
            ut_t.append(t)
        c2_group(grp)
    close_pool(ep)

    # ---- stage F': out[i,o] = (1/rs[i]) * sum_h UT[h,i] M2[h,o] ----
    for oc in range(H // 512):
        ps = [pp.tile([P, 512], F32, name="ps", tag="ps") for _ in range(8)]
        for mt in range(HT):
            wt = wsp.tile([P, 512], F16, name="wsf", tag="wsf")
            nc.sync.dma_start(
                out=wt[:], in_=m2[mt * P:(mt + 1) * P, oc * 512:(oc + 1) * 512])
            for isub in range(8):
                nc.tensor.matmul(ps[isub][:], ut_t[mt][:, isub * P:(isub + 1) * P],
                                 wt[:], start=(mt == 0), stop=(mt == HT - 1))
        for isub in range(8):
            t = osb.tile([P, 512], F32, name="osb", tag="osb")
            if isub % 2:
                nc.scalar.activation(t[:], ps[isub][:], ACT.Copy,
                                     scale=recip[:, isub:isub + 1])
            else:
                nc.vector.tensor_scalar_mul(t[:], ps[isub][:],
                                            recip[:, isub:isub + 1])
            nc.sync.dma_start(
                out=out[isub * P:(isub + 1) * P, oc * 512:(oc + 1) * 512],
                in_=t[:])
    close_pool(utp)

    for p in (rcp, osb, cp, wsp, pp):
        close_pool(p)


_NC_CACHE = {}


def build_nc(num_heads=16):
    key = int(num_heads)
    if key in _NC_CACHE:
        return _NC_CACHE[key]
    scale = 1.0 / float(np.sqrt(H // key))
    nc = bacc.Bacc("TRN2", target_bir_lowering=False, debug=False,
                   num_devices=NCORE)
    g = nc.dram_tensor("g", [H, H], F16, kind="ExternalInput").ap()
    m2 = nc.dram_tensor("m2", [H, H], F16, kind="ExternalInput").ap()
    hsqT = nc.dram_tensor("hsqT", [H, QB], F16, kind="ExternalInput").ap()
    hsT = nc.dram_tensor("hsT", [H, S], F16, kind="ExternalInput").ap()
    hs = nc.dram_tensor("hs", [S, H], F16, kind="ExternalInput").ap()
    out = nc.dram_tensor("out", [QB, H], F32, kind="ExternalOutput").ap()
    with tile.TileContext(nc) as tc:
        _emit(tc, g, m2, hsqT, hsT, hs, out, scale)
    nc.compile()
    _NC_CACHE[key] = nc
    return nc


def make_in_maps(hidden_states, wq, wk, wv, wo):
    hs_f = np.asarray(hidden_states, dtype=np.float32)
    wq32 = np.asarray(wq, np.float32)
    wk32 = np.asarray(wk, np.float32)
    wv32 = np.asarray(wv, np.float32)
    wo32 = np.asarray(wo, np.float32)
    # load-time weight fusion: G = Wq^T Wk (q.k^T = hs G hs^T),
    # M2 = Wv^T Wo^T (P hs Wv^T Wo^T = (P hs) M2)
    g16 = np.ascontiguousarray((wq32.T @ wk32).astype(np.float16))
    m216 = np.ascontiguousarray((wv32.T @ wo32.T).astype(np.float16))
    per_batch = {}
    for b in range(B):
        hsb = hs_f[b]
        per_batch[b] = (
            np.ascontiguousarray(hsb.T.astype(np.float16)),   # hsT
            np.ascontiguousarray(hsb.astype(np.float16)),     # hs
        )
    in_maps = []
    for c in range(NCORE):
        b, qb = divmod(c, NCORE // B)
        hsbT16, hsb16 = per_batch[b]
        in_maps.append({
            "g": g16,
            "m2": m216,
            "hsqT": np.ascontiguousarray(hsbT16[:, qb * QB:(qb + 1) * QB]),
            "hsT": hsbT16,
            "hs": hsb16,
        })
    return in_maps


def assemble(results):
    out = np.empty((B, S, H), dtype=np.float32)
    for c in range(NCORE):
        b, qb = divmod(c, NCORE // B)
        out[b, qb * QB:(qb + 1) * QB] = results[c]["out"]
    return out


def kernel(hidden_states, freqs_angle, wq, wk, wv, wo, num_heads):
    nc = build_nc(int(num_heads))
    in_maps = make_in_maps(hidden_states, wq, wk, wv, wo)
    res = run_bass_kernel_spmd(nc, in_maps, list(range(NCORE)))
    return assemble(res.results)


# revision 4
# speedup vs baseline: 1.0112x; 1.0012x over previous
"""Trainium2 Bass kernel for nn_CustomAttentionLayer (B=2, S=4096, H=2048), v4.

Math: RoPE here uses a position-independent angle vector, so the rotation is a
constant orthogonal transform applied to both q and k and cancels in
q.k^T (v is never rotated).  The layer reduces to (per batch):

    S   = hs Wq^T Wk hs^T * scale
    P   = softmax(S)
    out = P hs Wv^T Wo^T

The weight-only products G = Wq^T Wk and M2 = Wv^T Wo^T are
input-independent, so they are fused on the host (the same kind of
load-time weight fusion a serving stack would do) and shipped to the device
in fp16.  Every per-activation FLOP stays on device.  Per core (1024 query
rows, single pass, all matmul operands fp16, 1 cycle/row, fp32 PSUM):

  B' : aqT[h',i] = sum_h  G[h,h']   * hsqT[h,i]    (512 mm, N=512)
  C  : ST[j,i]   = sum_h' hsT[h',j] * aqT[h',i]    (1024 mm, N=512)
       expST = exp(scale*ST - 12)   (fp16; shift cancels in softmax)
  C2 : rs[i]     = sum_j  expST[j,i]   (ones-matmuls, interleaved into D)
  D  : UT[h,i]   = sum_j  hs[j,h]   * expST[j,i]   (1024 mm, N=512)
  F' : out[i,o]  = (1/rs[i]) sum_h UT[h,i]*M2[h,o] (512 mm, N=512)

3072 N=512 matmuls = 1.573M PE cycles/core (vs 2.118M for the direct
q/k/v/o projection form).  Zero collectives; every DRAM tensor streams
once; DMA demand ~60 GB/s per queue, far under the PE-bound span.
Sharding: core c = (batch c//4) x (query block c%4).
"""

import numpy as np

import concourse.bacc as bacc
import concourse.mybir as mybir
import concourse.tile as tile
from concourse.bass_utils import run_bass_kernel_spmd

F32 = mybir.dt.float32
F16 = mybir.dt.float16

B, S, H = 2, 4096, 2048
NCORE = 8
QB = (B * S) // NCORE  # 1024 query rows per core
P = 128
IC = QB
NI = IC // 512         # i-halves per output tile (N=512 each)
HT = H // P            # 16 tiles along any H-sized dim
JT = S // P            # 32 tiles along keys
EXP_SHIFT = -12.0      # exp(scale*logit - 12): max logit ~18 -> exp <= e^6


def _emit(tc, g, m2, hsqT, hsT, hs, out, scale):
    nc = tc.nc
    ACT = mybir.ActivationFunctionType

    cms = {}

    def open_pool(**kw):
        cm = tc.tile_pool(**kw)
        pool = cm.__enter__()
        cms[id(pool)] = cm
        return pool

    def close_pool(pool):
        cms.pop(id(pool)).__exit__(None, None, None)

    pp = open_pool(name="psum", bufs=8, space="PSUM")
    wsp = open_pool(name="wstream", bufs=20)
    cp = open_pool(name="const", bufs=1)
    osb = open_pool(name="outsb", bufs=4)
    rcp = open_pool(name="recip", bufs=1)

    ones = cp.tile([P, 1], F16, name="ones", tag="ones")
    nc.any.memset(ones[:], 1.0)
    expbias = cp.tile([P, 1], F32, name="expbias", tag="expbias")
    nc.any.memset(expbias[:], EXP_SHIFT)

    # PE warm-up: ~4us of tiny matmuls inside the initial DMA-fill window,
    # so the HAM clock gate reaches K=8/8 before stage B' starts.
    warm = cp.tile([P, P], F16, name="warm", tag="warm")
    nc.any.memset(warm[:], 0.0)
    wps = pp.tile([P, 512], F32, name="wps", tag="ps")
    for _ in range(176):
        nc.tensor.matmul(wps[:, 0:1], warm[:], ones[:],
                         start=True, stop=True)

    def evac_plain(dst, ps, k):
        (nc.scalar.copy if k % 2 else nc.vector.tensor_copy)(dst, ps[:])

    # ---- hsqT into SBUF (B' rhs) ----
    hsqp = open_pool(name="hsq", bufs=HT, side="left")
    hsq_t = []
    for ht in range(HT):
        t = hsqp.tile([P, IC], F16, name="hsq", tag="hsq")
        nc.gpsimd.dma_start(out=t[:], in_=hsqT[ht * P:(ht + 1) * P, :])
        hsq_t.append(t)

    # ---- stage B': aqT[h',i] = sum_h G[h,h'] hsqT[h,i] ----
    aqp = open_pool(name="aqT", bufs=HT, side="right")
    aq_t = []
    k = 0
    for pair in range(8):              # h'-tile pairs
        ps = [pp.tile([P, 512], F32, name="ps", tag="ps") for _ in range(4)]
        for ht in range(HT):
            wt = wsp.tile([P, 2 * P], F16, name="wsb", tag="wsb")
            nc.sync.dma_start(
                out=wt[:],
                in_=g[ht * P:(ht + 1) * P, pair * 2 * P:(pair + 1) * 2 * P])
            for jj in range(2):
                for ih in range(NI):
                    nc.tensor.matmul(
                        ps[jj * NI + ih][:], wt[:, jj * P:(jj + 1) * P],
                        hsq_t[ht][:, ih * 512:(ih + 1) * 512],
                        start=(ht == 0), stop=(ht == HT - 1))
        for jj in range(2):
            t = aqp.tile([P, IC], F16, name="aqT", tag="aqT")
            for ih in range(NI):
                evac_plain(t[:, ih * 512:(ih + 1) * 512], ps[jj * NI + ih], k)
                k += 1
            aq_t.append(t)
    close_pool(hsqp)

    # ---- stage C: expST[j,i] = exp(scale*ST - 12) ----
    ep = open_pool(name="expST", bufs=JT, side="left")
    exp_t = []
    for jg in range(JT // 2):
        ps = [pp.tile([P, 512], F32, name="ps", tag="ps") for _ in range(4)]
        for ht in range(HT):
            kt = wsp.tile([P, 2 * P], F16, name="wsc", tag="wsc")
            nc.sync.dma_start(
                out=kt[:],
                in_=hsT[ht * P:(ht + 1) * P, jg * 2 * P:(jg + 1) * 2 * P])
            for jj in range(2):
                for ih in range(NI):
                    nc.tensor.matmul(
                        ps[jj * NI + ih][:], kt[:, jj * P:(jj + 1) * P],
                        aq_t[ht][:, ih * 512:(ih + 1) * 512],
                        start=(ht == 0), stop=(ht == HT - 1))
        for jj in range(2):
            t = ep.tile([P, IC], F16, name="expST", tag="expST")
            for ih in range(NI):
                nc.scalar.activation(t[:, ih * 512:(ih + 1) * 512],
                                     ps[jj * NI + ih][:], ACT.Exp,
                                     scale=scale, bias=expbias[:])
            exp_t.append(t)
    close_pool(aqp)

    recip = rcp.tile([P, IC // P], F32, name="recip", tag="recip")

    def c2_group(isub):
        # rowsum of expST for one 128-query block; interleaved into stage D
        # so the PE never idles long enough to re-throttle.
        prs = pp.tile([P, 1], F32, name="psr", tag="ps")
        for jt in range(JT):
            nc.tensor.matmul(prs[:], exp_t[jt][:, isub * P:(isub + 1) * P],
                             ones[:], start=(jt == 0), stop=(jt == JT - 1))
        nc.vector.reciprocal(recip[:, isub:isub + 1], prs[:])

    # ---- stage D: UT[h,i] = sum_j hs[j,h] expST[j,i] (C2 interleaved) ----
    utp = open_pool(name="UT", bufs=HT, side="right")
    ut_t = []
    k = 0
    for grp in range(8):               # 2 h-tiles per group
        ps = [pp.tile([P, 512], F32, name="ps", tag="ps") for _ in range(4)]
        for jt in range(JT):
            wt = wsp.tile([P, 2 * P], F16, name="wsd", tag="wsd")
            nc.gpsimd.dma_start(
                out=wt[:],
                in_=hs[jt * P:(jt + 1) * P, grp * 2 * P:(grp + 1) * 2 * P])
            for mm in range(2):
                for ih in range(NI):
                    nc.tensor.matmul(
                        ps[mm * NI + ih][:], wt[:, mm * P:(mm + 1) * P],
                        exp_t[jt][:, ih * 512:(ih + 1) * 512],
                        start=(jt == 0), stop=(jt == JT - 1))
        for mm in range(2):
            t = utp.tile([P, IC], F16, name="UT", tag="UT")
            for ih in range(NI):
                evac_plain(t[:, ih * 512:(ih + 1) * 512], ps[mm * NI + ih], k)
                k += 1
            ut_t.append(t)
        c2_group(grp)
    close_pool(ep)

    # ---- stage F': out[i,o] = (1/rs[i]) * sum_h UT[h,i] M2[h,o] ----
    # Two 4-psum halves per oc group keep two psum groups in flight (full
    # double-buffering); the m2 weight tiles stay in SBUF across both halves.
    for oc in range(H // 512):
        wts = []
        for mt in range(HT):
            wt = wsp.tile([P, 512], F16, name="wsf", tag="wsf")
            nc.sync.dma_start(
                out=wt[:], in_=m2[mt * P:(mt + 1) * P, oc * 512:(oc + 1) * 512])
            wts.append(wt)
        for half in range(2):
            ps = [pp.tile([P, 512], F32, name="ps", tag="ps") for _ in range(4)]
            for mt in range(HT):
                for i4 in range(4):
                    isub = half * 4 + i4
                    nc.tensor.matmul(
                        ps[i4][:], ut_t[mt][:, isub * P:(isub + 1) * P],
                        wts[mt][:], start=(mt == 0), stop=(mt == HT - 1))
            for i4 in range(4):
                isub = half * 4 + i4
                t = osb.tile([P, 512], F32, name="osb", tag="osb")
                if isub % 2:
                    nc.scalar.activation(t[:], ps[i4][:], ACT.Copy,
                                         scale=recip[:, isub:isub + 1])
                else:
                    nc.vector.tensor_scalar_mul(t[:], ps[i4][:],
                                                recip[:, isub:isub + 1])
                nc.sync.dma_start(
                    out=out[isub * P:(isub + 1) * P, oc * 512:(oc + 1) * 512],
                    in_=t[:])
    close_pool(utp)

    for p in (rcp, osb, cp, wsp, pp):
        close_pool(p)


_NC_CACHE = {}


def build_nc(num_heads=16):
    key = int(num_heads)
    if key in _NC_CACHE:
        return _NC_CACHE[key]
    scale = 1.0 / float(np.sqrt(H // key))
    nc = bacc.Bacc("TRN2", target_bir_lowering=False, debug=False,
                   num_devices=NCORE)
    g = nc.dram_tensor("g", [H, H], F16, kind="ExternalInput").ap()
    m2 = nc.dram_tensor("m2", [H, H], F16, kind="ExternalInput").ap()
    hsqT = nc.dram_tensor("hsqT", [H, QB], F16, kind="ExternalInput").ap()
    hsT = nc.dram_tensor("hsT", [H, S], F16, kind="ExternalInput").ap()
    hs = nc.dram_tensor("hs", [S, H], F16, kind="ExternalInput").ap()
    out = nc.dram_tensor("out", [QB, H], F32, kind="ExternalOutput").ap()
    with tile.TileContext(nc) as tc:
        _emit(tc, g, m2, hsqT, hsT, hs, out, scale)
    nc.compile()
    _NC_CACHE[key] = nc
    return nc


def make_in_maps(hidden_states, wq, wk, wv, wo):
    hs_f = np.asarray(hidden_states, dtype=np.float32)
    wq32 = np.asarray(wq, np.float32)
    wk32 = np.asarray(wk, np.float32)
    wv32 = np.asarray(wv, np.float32)
    wo32 = np.asarray(wo, np.float32)
    # load-time weight fusion: G = Wq^T Wk (q.k^T = hs G hs^T),
    # M2 = Wv^T Wo^T (P hs Wv^T Wo^T = (P hs) M2)
    g16 = np.ascontiguousarray((wq32.T @ wk32).astype(np.float16))
    m216 = np.ascontiguousarray((wv32.T @ wo32.T).astype(np.float16))
    per_batch = {}
    for b in range(B):
        hsb = hs_f[b]
        per_batch[b] = (
            np.ascontiguousarray(hsb.T.astype(np.float16)),   # hsT
            np.ascontiguousarray(hsb.astype(np.float16)),     # hs
        )
    in_maps = []
    for c in range(NCORE):
        b, qb = divmod(c, NCORE // B)
        hsbT16, hsb16 = per_batch[b]
        in_maps.append({
            "g": g16,
            "m2": m216,
            "hsqT": np.ascontiguousarray(hsbT16[:, qb * QB:(qb + 1) * QB]),
            "hsT": hsbT16,
            "hs": hsb16,
        })
    return in_maps


def assemble(results):
    out = np.empty((B, S, H), dtype=np.float32)
    for c in range(NCORE):
        b, qb = divmod(c, NCORE // B)
        out[b, qb * QB:(qb + 1) * QB] = results[c]["out"]
    return out


def kernel(hidden_states, freqs_angle, wq, wk, wv, wo, num_heads):
    nc = build_nc(int(num_heads))
    in_maps = make_in_maps(hidden_states, wq, wk, wv, wo)
    res = run_bass_kernel_spmd(nc, in_maps, list(range(NCORE)))
    return assemble(res.results)
